# revision 17
# baseline (speedup 1.0000x reference)
"""Trainium2 Bass kernel for the show-attend-tell style attention module.

  att_h   = h @ W_h2att.T + b_h2att                      # [B, H]
  dot     = tanh(p_att_feats + att_h[:, None, :])        # [B, S, H]
  scores  = dot @ w_alpha + b_alpha                      # [B, S]
  weight  = softmax(scores) * mask, renormalized         # [B, S]
  att_res = sum_s weight[:, s] * att_feats[:, s, :]      # [B, D]

B=256, S=196, D=2048, H=512.  Data-parallel over 8 NeuronCores (32
batches per core); params replicated.  b_alpha cancels inside softmax
and is ignored.  The mask renorm is fused into the softmax denominator:
weight = exp(s - max) * mask / (sum(exp(s - max) * mask) + eps), which
equals the reference's softmax -> mask -> renormalize chain exactly
(the first softmax's denominator cancels; eps only guards all-masked
rows).

Memory-regime problem: att_feats and p_att_feats are streamed in
float16 (halves HBM traffic vs fp32; fp16 quantization error ~5e-4 is
far inside the 2e-2 gate).  Per 4-batch quartet the kernel issues two
input DMAs and one output DMA:

  - att_feats as ONE full 128-partition gpsimd/SWDGE transfer with
    32 KB lines (partition p carries s-rows p and 128+p, s zero-padded
    to 256 host-side).  Partial-partition DMAs fall off a cliff on this
    hardware (HW-measured: 80 GB/s for 68 partitions vs 375-380 GB/s
    for 128), so padding 30% more bytes is a large net win.
  - p_att_feats as one [128, 6.3 KB-line] transfer on the scalar HWDGE
    queue, overlapping the feats stream.

The sync HWDGE queue is reserved for the tiny output stores: mixing
the large feats loads onto it (or fusing everything onto one queue)
measured significantly slower.

Batches are processed in quartets.  Scores for batch q land on PSUM
partition 32q (matmul M=1 base-partition rule).  The masked softmax
runs batched over the [128, 224] score tile (mask rows != 32q are zero
host-side, so junk rows produce zero weights).  A single matmul against
a host-prepared selection matrix Msel (Msel[32q, 4q] = 1) transposes
and zero-interleaves the weights in one shot:

  wtT[s, j] = sum_r wgt[r, s] * Msel[r, j]  ->  col 4q = batch q weights

Phase B then accumulates all 4 batches of a quartet into one [4, 512]
PSUM region: the matmul for batch q uses lhsT = wtT[:, 3q:3q+4], whose
col q is batch q's weights and whose other cols are exactly zero, so
row j accumulates only batch j's weighted sum.  This makes the
PSUM->SBUF copies contiguous [4, 2, 512] and the store one [4, 2048]
DMA per quartet.
"""

import sys

if "/opt/trn_rl_repo" not in sys.path:
    sys.path.insert(0, "/opt/trn_rl_repo")

from contextlib import ExitStack

import numpy as np

import concourse.bacc as bacc
import concourse.tile as tile
from concourse import mybir
from concourse.bass_utils import run_bass_kernel_spmd

# Problem dims (hardcoded per the harness contract).
B, S, D, H = 256, 196, 2048, 512
P = 128          # partitions
HC = H // P      # 4 h-chunks
DC = D // P      # 16 d-chunks
S0 = 128         # first s-chunk rows
S1 = S - S0      # second s-chunk rows (68)
SP = 224         # padded s for the softmax tile (mask cols >= S are 0)
G = 4            # batches per quartet
N_CORES = 8
BS = B // N_CORES  # 32 batches per core
NG = BS // G       # 8 quartets per core

FP32 = mybir.dt.float32
FP16 = mybir.dt.float16
AX = mybir.AxisListType
AF = mybir.ActivationFunctionType


def build_program(bs=BS, reps=1, fbufs=4, pbufs=3, wbufs=3):
    """Build the single-core Bass/Tile program (SPMD across cores)."""
    nc = bacc.Bacc("TRN2", target_bir_lowering=False, debug=False)

    assert bs % G == 0
    ngroups = bs // G

    featsq = nc.dram_tensor("featsq", [ngroups, P, 2, G, D], FP16, kind="ExternalInput").ap()
    pTq = nc.dram_tensor("pTq", [ngroups, P, G, HC * S], FP16, kind="ExternalInput").ap()
    hT = nc.dram_tensor("hT", [P, DC, bs], FP16, kind="ExternalInput").ap()
    WT = nc.dram_tensor("WT", [P, DC, H], FP16, kind="ExternalInput").ap()
    wal = nc.dram_tensor("walpha", [P, HC], FP16, kind="ExternalInput").ap()
    bh = nc.dram_tensor("bh", [1, H], FP16, kind="ExternalInput").ap()
    masksT = nc.dram_tensor("masksT", [P, NG, SP], FP32, kind="ExternalInput").ap()
    msel = nc.dram_tensor("msel", [P, 16], FP16, kind="ExternalInput").ap()
    out = nc.dram_tensor("out", [bs, D], FP32, kind="ExternalOutput").ap()

    with tile.TileContext(nc) as tc, ExitStack() as ctx:
        singles = ctx.enter_context(tc.tile_pool(name="singles", bufs=1))
        f1pool = ctx.enter_context(tc.tile_pool(name="f1pool", bufs=fbufs))
        ppool = ctx.enter_context(tc.tile_pool(name="ppool", bufs=pbufs))
        apool = ctx.enter_context(tc.tile_pool(name="apool", bufs=2))
        gpool = ctx.enter_context(tc.tile_pool(name="gpool", bufs=2))
        wtpool = ctx.enter_context(tc.tile_pool(name="wtpool", bufs=wbufs))
        rpool = ctx.enter_context(tc.tile_pool(name="rpool", bufs=2))
        ps_sc = ctx.enter_context(tc.tile_pool(name="ps_sc", bufs=2, space="PSUM"))
        ps_wt = ctx.enter_context(tc.tile_pool(name="ps_wt", bufs=1, space="PSUM"))
        ps_res = ctx.enter_context(tc.tile_pool(name="ps_res", bufs=2, space="PSUM"))

        # ---- constants / params (preamble, outside the rep loop) ----
        ht_sb = singles.tile([P, DC, bs], FP16)
        nc.gpsimd.dma_start(out=ht_sb, in_=hT)
        wt_sb = singles.tile([P, DC, H], FP16)
        nc.gpsimd.dma_start(out=wt_sb, in_=WT)
        wal_sb = singles.tile([P, HC], FP16)
        nc.gpsimd.dma_start(out=wal_sb, in_=wal)
        bh_sb = singles.tile([1, H], FP16)
        nc.gpsimd.dma_start(out=bh_sb, in_=bh)
        msk_sb = singles.tile([P, NG, SP], FP32)
        nc.gpsimd.dma_start(out=msk_sb, in_=masksT)
        msel_sb = singles.tile([P, 16], FP16)
        nc.gpsimd.dma_start(out=msel_sb, in_=msel)
        ones_sb = singles.tile([1, bs], FP16)
        nc.vector.memset(ones_sb, 1.0)

        # ---- att_h = W @ h^T + b  ->  atth_sb [P, HC, bs] fp16 ----
        atth_sb = singles.tile([P, HC, bs], FP32)
        for hc in range(HC):
            a_ps_full = ps_sc.tile([P, SP], FP32, tag="sc")
            a_ps = a_ps_full[:, 0:bs]
            for dc in range(DC):
                nc.tensor.matmul(
                    a_ps,
                    lhsT=wt_sb[:, dc, hc * P : (hc + 1) * P],
                    rhs=ht_sb[:, dc, :],
                    start=(dc == 0),
                    stop=False,
                )
            nc.tensor.matmul(
                a_ps,
                lhsT=bh_sb[:, hc * P : (hc + 1) * P],
                rhs=ones_sb,
                start=False,
                stop=True,
            )
            nc.vector.tensor_copy(out=atth_sb[:, hc, :], in_=a_ps)

        def load_f(g):
            """Kick off the quartet's att_feats DMA (fp16, gpsimd/SWDGE).

            One full 128-partition transfer with 32 KB lines: partition p
            carries s-rows p and 128+p (s zero-padded to 256 host-side).
            Partial-partition DMAs run at a fraction of line rate (HW: 80
            GB/s for 68 partitions vs 375-380 GB/s for 128 with 16-32 KB
            lines), so padding 30% more bytes is a large net win, and the
            single fused DMA costs one SWDGE descriptor-emission instead
            of two.
            """
            gg = g % ngroups
            f = f1pool.tile([P, 2, G, D], FP16)
            nc.gpsimd.dma_start(out=f, in_=featsq[gg])
            return f

        def load_p(g):
            """Kick off the quartet's p_att_feats DMA (fp16, scalar queue)."""
            gg = g % ngroups
            p_sb = ppool.tile([P, G, HC * S], FP16)
            nc.scalar.dma_start(out=p_sb, in_=pTq[gg])
            return p_sb

        def a_scores(g, p_sb):
            """Bias-add + tanh + score matmuls for quartet g (scores land on
            PSUM partition 32q).  Returns the score PSUM tile."""
            sc_ps = ps_sc.tile([P, SP], FP32, tag="sc")
            nc.vector.memset(sc_ps, 0.0)
            for q in range(G):
                b = (g * G + q) % bs
                poff = 32 * q
                padd = apool.tile([P, HC, S], FP16, tag="padd")
                for hc in range(HC):
                    nc.vector.tensor_scalar_add(
                        padd[:, hc, :],
                        p_sb[:, q, hc * S : (hc + 1) * S],
                        atth_sb[:, hc, b : b + 1],
                    )
                dot = apool.tile([P, HC, S], FP16, tag="dot")
                nc.scalar.activation(out=dot, in_=padd, func=AF.Tanh, scale=1.0)
                for hc in range(HC):
                    nc.tensor.matmul(
                        sc_ps[poff : poff + 1, 0:S],
                        lhsT=wal_sb[:, hc : hc + 1],
                        rhs=dot[:, hc, :],
                        start=(hc == 0),
                        stop=(hc == HC - 1),
                        tile_position=(0, poff),
                    )
            return sc_ps

        def a_finish(g, sc_ps):
            """Masked softmax + select-transpose for quartet g.

            Returns wtT_sb [P, 2, 16] fp16: [:, 0, 4q] = batch q weights for
            s in [0,128); [:, 1, 4q] = weights for s in [128,196); all other
            columns exactly zero (mask rows != 32q are zero host-side).
            """
            gg = g % ngroups
            mx = gpool.tile([P, 1], FP32)
            nc.vector.reduce_max(mx, sc_ps, axis=AX.X)
            nm = gpool.tile([P, 1], FP32)
            nc.vector.tensor_scalar_mul(nm, mx, -1.0)
            e_sb = gpool.tile([P, SP], FP32)
            nc.scalar.activation(out=e_sb, in_=sc_ps, func=AF.Exp, bias=nm, scale=1.0)
            em = gpool.tile([P, SP], FP32)
            nc.vector.tensor_mul(em, e_sb, msk_sb[:, gg, :])
            zz = gpool.tile([P, 1], FP32)
            nc.vector.reduce_sum(zz, em, axis=AX.X)
            zze = gpool.tile([P, 1], FP32)
            nc.vector.tensor_scalar_add(zze, zz, 1e-20)
            rz = gpool.tile([P, 1], FP32)
            nc.vector.reciprocal(rz, zze)
            wgt = gpool.tile([P, SP], FP16)
            nc.vector.tensor_scalar_mul(wgt, em, rz)

            # select-transpose: wtT[s, j] = sum_r wgt[r, s] * msel[r, j]
            wtT_ps = ps_wt.tile([P, 2, 16], FP32)
            nc.tensor.matmul(
                wtT_ps[:, 0, :], lhsT=wgt[:, 0:P], rhs=msel_sb, start=True, stop=True
            )
            nc.tensor.matmul(
                wtT_ps[0 : SP - P, 1, :],
                lhsT=wgt[:, P:SP],
                rhs=msel_sb,
                start=True,
                stop=True,
            )
            wtT_sb = wtpool.tile([P, 2, 16], FP16)
            nc.vector.tensor_copy(out=wtT_sb[:, 0, :], in_=wtT_ps[:, 0, :])
            nc.vector.tensor_copy(
                out=wtT_sb[0 : SP - P, 1, :], in_=wtT_ps[0 : SP - P, 1, :]
            )
            return wtT_sb

        def phase_b_mm(g, wtT_sb, f):
            """att_res rows for quartet g: 4 batches accumulate into one
            [4, 512] PSUM region per column chunk via zero-interleaved
            weight windows wtT[:, 3q:3q+4].  The half-0 PSUM->SBUF copy
            (DVE) overlaps half-1's matmuls; the half-1 copy is deferred
            to phase_b_fin one iteration later so it never blocks the next
            quartet's prep work in the engine FIFOs."""
            row_sb = rpool.tile([G, D // 512, 512], FP32)
            res_list = []
            for half in range(2):
                res_ps = ps_res.tile([G, 2, 512], FP32)
                res_list.append(res_ps)
                for q in range(G):
                    for sc in range(2):
                        lhsT = (
                            wtT_sb[:, 0, 3 * q : 3 * q + 4]
                            if sc == 0
                            else wtT_sb[0:S1, 1, 3 * q : 3 * q + 4]
                        )
                        for c in range(2):
                            cc = 2 * half + c
                            rhs = (
                                f[:, 0, q, cc * 512 : (cc + 1) * 512]
                                if sc == 0
                                else f[0:S1, 1, q, cc * 512 : (cc + 1) * 512]
                            )
                            nc.tensor.matmul(
                                res_ps[:, c, :],
                                lhsT=lhsT,
                                rhs=rhs,
                                start=(q == 0 and sc == 0),
                                stop=(q == G - 1 and sc == 1),
                            )
                if half == 0:
                    nc.vector.tensor_copy(out=row_sb[:, 0:2, :], in_=res_ps)
            return row_sb, res_list[1]

        def phase_b_fin(g, row_sb, res_ps1):
            """Trailing half-1 copy (scalar engine) + output store (sync
            HWDGE, off the critical gpsimd input queue)."""
            gg = g % ngroups
            nc.scalar.copy(out=row_sb[:, 2:4, :], in_=res_ps1)
            nc.sync.dma_start(out=out[gg * G : (gg + 1) * G, :], in_=row_sb)

        # Software pipeline (loads lead by 2 quartets, phase A by 1):
        #   iteration it emits
        #     loads(it+2) | a_scores(it+1) | b_fin(it-1) | b_mm(it) | a_finish(it+1)
        # so the PE FIFO is [scores(it+1)][B(it)][selects(it+1)][scores(it+2)]..
        # with every operand prepared one iteration ahead, and the trailing
        # PSUM copy of B(it-1) lands on an idle engine slot instead of
        # blocking the next quartet's bias-adds/tanh.
        total = reps * ngroups
        fs = {0: load_f(0), 1: load_f(1)}
        ps = {0: load_p(0), 1: load_p(1)}
        wts = {0: a_finish(0, a_scores(0, ps.pop(0)))}
        fin = {}
        for it in range(total):
            if it + 2 < total:
                fs[it + 2] = load_f(it + 2)
                ps[it + 2] = load_p(it + 2)
            if it + 1 < total:
                sc = a_scores(it + 1, ps.pop(it + 1))
            if it >= 1:
                phase_b_fin(it - 1, *fin.pop(it - 1))
            fin[it] = phase_b_mm(it, wts.pop(it), fs.pop(it))
            if it + 1 < total:
                wts[it + 1] = a_finish(it + 1, sc)
        phase_b_fin(total - 1, *fin.pop(total - 1))

    nc.compile()
    return nc


def host_prepare(inputs, bs=BS):
    """Pre-layout full inputs into per-core in_maps (host-side, untimed)."""
    h = np.asarray(inputs["h"], dtype=np.float32)
    att_feats = np.asarray(inputs["att_feats"], dtype=np.float32)
    p = np.asarray(inputs["p_att_feats"], dtype=np.float32)
    att_masks = np.asarray(inputs["att_masks"], dtype=np.float32)
    W = np.asarray(inputs["W_h2att"], dtype=np.float32)
    b_h2att = np.asarray(inputs["b_h2att"], dtype=np.float32)
    w_alpha = np.asarray(inputs["w_alpha"], dtype=np.float32)

    n_cores = h.shape[0] // bs
    ng = bs // G

    # [P, DC, H]: WT[p, dc, h] = W[h, dc*P + p]
    WT = np.ascontiguousarray(
        W.reshape(H, DC, P).transpose(2, 1, 0).astype(np.float16)
    )
    # [P, HC]: wal[p, hc] = w_alpha[hc*P + p]
    wal = np.ascontiguousarray(w_alpha.reshape(HC, P).T.astype(np.float16))
    bhv = np.ascontiguousarray(b_h2att.reshape(1, H).astype(np.float16))
    # selection matrix: Msel[32q, 4q] = 1
    msel = np.zeros((P, 16), dtype=np.float16)
    for q in range(G):
        msel[32 * q, 4 * q] = 1.0

    in_maps = []
    for c in range(n_cores):
        b0 = c * bs
        # [ng, P, 2, G, D] fp16: partition p carries s-rows p and 128+p
        # (s zero-padded to 256) so the per-quartet feats DMA is one full
        # 128-partition transfer with 32 KB lines
        fq = np.zeros((ng, 2 * P, G, D), dtype=np.float16)
        fq[:, 0:S] = (
            att_feats[b0 : b0 + bs]
            .reshape(ng, G, S, D)
            .transpose(0, 2, 1, 3)
            .astype(np.float16)
        )
        fq = np.ascontiguousarray(fq.reshape(ng, 2, P, G, D).transpose(0, 2, 1, 3, 4))
        # [ng, P, G, HC*S] fp16: pTq[g, p, q, hc*S + s] = p[g*G+q, s, hc*P+p]
        pTq = np.ascontiguousarray(
            p[b0 : b0 + bs]
            .reshape(ng, G, S, HC, P)
            .transpose(0, 4, 1, 3, 2)
            .reshape(ng, P, G, HC * S)
            .astype(np.float16)
        )
        # [P, DC, bs] fp16: hT[p, dc, b] = h[b, dc*P + p]
        hT = np.ascontiguousarray(
            h[b0 : b0 + bs].reshape(bs, DC, P).transpose(2, 1, 0).astype(np.float16)
        )
        # [P, ng, SP] fp32: row 32q of group g = mask[g*G+q] (padded), else 0
        mT = np.zeros((P, ng, SP), dtype=np.float32)
        for q in range(G):
            mT[32 * q, :, 0:S] = att_masks[b0 : b0 + bs].reshape(ng, G, S)[:, q, :]
        in_maps.append(
            {
                "featsq": fq,
                "pTq": pTq,
                "hT": hT,
                "WT": WT,
                "walpha": wal,
                "bh": bhv,
                "masksT": np.ascontiguousarray(mT),
                "msel": msel,
            }
        )
    return in_maps


_PROGRAM = None


def _get_program():
    global _PROGRAM
    if _PROGRAM is None:
        _PROGRAM = build_program()
    return _PROGRAM


def run(inputs, trace=False):
    nc = _get_program()
    in_maps = host_prepare(inputs)
    res = run_bass_kernel_spmd(nc, in_maps, list(range(N_CORES)), trace=trace)
    out = np.concatenate([r["out"] for r in res.results], axis=0)
    return out, res


def kernel(**inputs) -> np.ndarray:
    out, _ = run(inputs, trace=False)
    return out


def _enable_ntff():
    """Best-effort: register the axon NTFF profile hook if the image's
    antenv package lacks axon_hooks.  Analysis/timing only."""
    import types

    name = "antenv.axon_hooks"
    if name in sys.modules:
        return True
    try:
        mod = types.ModuleType(name)
        state = {"hook": None}
        mod.set_axon_ntff_profile_hook = lambda h: state.__setitem__("hook", h)
        mod.get_axon_ntff_profile_hook = lambda: state["hook"]
        sys.modules[name] = mod
        import antenv

        antenv.axon_hooks = mod
        from trn_agent_boot.trn_boot import _ntff_profile_via_ctypes

        mod.set_axon_ntff_profile_hook(
            _ntff_profile_via_ctypes("/opt/axon/libaxon_pjrt.so")
        )
        return True
    except Exception:
        sys.modules.pop(name, None)
        return False


def bench(inputs, reps_long=5):
    """Device-measured timing via NTFF profiles: run the reps=1 and
    reps=reps_long programs with tracing and difference their on-device
    exec times.  The preamble (param loads + att_h) and profile framing
    cancel in the slope; the result is the steady-state time for one
    pass over the data.

    Returns (per_rep_s, t1_s, tn_s, out).
    """
    have_ntff = _enable_ntff()
    in_maps = host_prepare(inputs)
    nc1 = _get_program()
    ncn = build_program(reps=reps_long)

    def timed(nc):
        res = run_bass_kernel_spmd(
            nc, in_maps, list(range(N_CORES)), trace=have_ntff
        )
        out = np.concatenate([r["out"] for r in res.results], axis=0)
        return res.exec_time_ns, out

    t1, out = timed(nc1)
    tn, _ = timed(ncn)
    if t1 is None or tn is None:
        raise RuntimeError("NTFF timing unavailable (no exec_time_ns)")
    per_rep = (tn - t1) / (reps_long - 1) * 1e-9
    return per_rep, t1 * 1e-9, tn * 1e-9, out


# revision 19
# speedup vs baseline: 1.0138x; 1.0138x over previous
"""Trainium2 Bass kernel for the show-attend-tell style attention module.

  att_h   = h @ W_h2att.T + b_h2att                      # [B, H]
  dot     = tanh(p_att_feats + att_h[:, None, :])        # [B, S, H]
  scores  = dot @ w_alpha + b_alpha                      # [B, S]
  weight  = softmax(scores) * mask, renormalized         # [B, S]
  att_res = sum_s weight[:, s] * att_feats[:, s, :]      # [B, D]

B=256, S=196, D=2048, H=512.  Data-parallel over 8 NeuronCores (32
batches per core); params replicated.  b_alpha cancels inside softmax
and is ignored.  The mask renorm is fused into the softmax denominator:
weight = exp(s - max) * mask / (sum(exp(s - max) * mask) + eps), which
equals the reference's softmax -> mask -> renormalize chain exactly
(the first softmax's denominator cancels; eps only guards all-masked
rows).

Memory-regime problem: att_feats and p_att_feats are streamed in
float16 (halves HBM traffic vs fp32; fp16 quantization error ~5e-4 is
far inside the 2e-2 gate).  Per 4-batch quartet the kernel issues two
input DMAs and one output DMA:

  - att_feats as ONE full 128-partition gpsimd/SWDGE transfer with
    32 KB lines (partition p carries s-rows p and 128+p, s zero-padded
    to 256 host-side).  Partial-partition DMAs fall off a cliff on this
    hardware (HW-measured: 80 GB/s for 68 partitions vs 375-380 GB/s
    for 128), so padding 30% more bytes is a large net win.
  - p_att_feats as one [128, 6.3 KB-line] transfer on the scalar HWDGE
    queue, overlapping the feats stream.

The sync HWDGE queue is reserved for the tiny output stores: mixing
the large feats loads onto it (or fusing everything onto one queue)
measured significantly slower.

Batches are processed in quartets.  Scores for batch q land on PSUM
partition 32q (matmul M=1 base-partition rule).  The masked softmax
runs batched over the [128, 224] score tile (mask rows != 32q are zero
host-side, so junk rows produce zero weights).  A single matmul against
a host-prepared selection matrix Msel (Msel[32q, 4q] = 1) transposes
and zero-interleaves the weights in one shot:

  wtT[s, j] = sum_r wgt[r, s] * Msel[r, j]  ->  col 4q = batch q weights

Phase B then accumulates all 4 batches of a quartet into one [4, 512]
PSUM region: the matmul for batch q uses lhsT = wtT[:, 3q:3q+4], whose
col q is batch q's weights and whose other cols are exactly zero, so
row j accumulates only batch j's weighted sum.  This makes the
PSUM->SBUF copies contiguous [4, 2, 512] and the store one [4, 2048]
DMA per quartet.
"""

import sys

if "/opt/trn_rl_repo" not in sys.path:
    sys.path.insert(0, "/opt/trn_rl_repo")

from contextlib import ExitStack

import numpy as np

import concourse.bacc as bacc
import concourse.tile as tile
from concourse import mybir
from concourse.bass_utils import run_bass_kernel_spmd

# Problem dims (hardcoded per the harness contract).
B, S, D, H = 256, 196, 2048, 512
P = 128          # partitions
HC = H // P      # 4 h-chunks
DC = D // P      # 16 d-chunks
S0 = 128         # first s-chunk rows
S1 = S - S0      # second s-chunk rows (68)
SP = 224         # padded s for the softmax tile (mask cols >= S are 0)
G = 4            # batches per quartet
N_CORES = 8
BS = B // N_CORES  # 32 batches per core
NG = BS // G       # 8 quartets per core

FP32 = mybir.dt.float32
FP16 = mybir.dt.float16
AX = mybir.AxisListType
AF = mybir.ActivationFunctionType


def build_program(bs=BS, reps=1, fbufs=4, pbufs=3, wbufs=3):
    """Build the single-core Bass/Tile program (SPMD across cores)."""
    nc = bacc.Bacc("TRN2", target_bir_lowering=False, debug=False)

    assert bs % G == 0
    ngroups = bs // G

    featsq = nc.dram_tensor("featsq", [ngroups, P, 2, G, D], FP16, kind="ExternalInput").ap()
    pTq = nc.dram_tensor("pTq", [ngroups, P, G, HC * S], FP16, kind="ExternalInput").ap()
    hT = nc.dram_tensor("hT", [P, DC, bs], FP16, kind="ExternalInput").ap()
    WT = nc.dram_tensor("WT", [P, DC, H], FP16, kind="ExternalInput").ap()
    wal = nc.dram_tensor("walpha", [P, HC], FP16, kind="ExternalInput").ap()
    bh = nc.dram_tensor("bh", [1, H], FP16, kind="ExternalInput").ap()
    masksT = nc.dram_tensor("masksT", [P, NG, SP], FP32, kind="ExternalInput").ap()
    msel = nc.dram_tensor("msel", [P, 16], FP16, kind="ExternalInput").ap()
    out = nc.dram_tensor("out", [bs, D], FP32, kind="ExternalOutput").ap()

    with tile.TileContext(nc) as tc, ExitStack() as ctx:
        singles = ctx.enter_context(tc.tile_pool(name="singles", bufs=1))
        f1pool = ctx.enter_context(tc.tile_pool(name="f1pool", bufs=fbufs))
        ppool = ctx.enter_context(tc.tile_pool(name="ppool", bufs=pbufs))
        apool = ctx.enter_context(tc.tile_pool(name="apool", bufs=2))
        gpool = ctx.enter_context(tc.tile_pool(name="gpool", bufs=2))
        wtpool = ctx.enter_context(tc.tile_pool(name="wtpool", bufs=wbufs))
        rpool = ctx.enter_context(tc.tile_pool(name="rpool", bufs=2))
        ps_sc = ctx.enter_context(tc.tile_pool(name="ps_sc", bufs=2, space="PSUM"))
        ps_wt = ctx.enter_context(tc.tile_pool(name="ps_wt", bufs=1, space="PSUM"))
        ps_res = ctx.enter_context(tc.tile_pool(name="ps_res", bufs=2, space="PSUM"))

        # ---- constants / params (preamble, outside the rep loop) ----
        ht_sb = singles.tile([P, DC, bs], FP16)
        nc.gpsimd.dma_start(out=ht_sb, in_=hT)
        wt_sb = singles.tile([P, DC, H], FP16)
        nc.gpsimd.dma_start(out=wt_sb, in_=WT)
        wal_sb = singles.tile([P, HC], FP16)
        nc.gpsimd.dma_start(out=wal_sb, in_=wal)
        bh_sb = singles.tile([1, H], FP16)
        nc.gpsimd.dma_start(out=bh_sb, in_=bh)
        msk_sb = singles.tile([P, NG, SP], FP32)
        nc.gpsimd.dma_start(out=msk_sb, in_=masksT)
        msel_sb = singles.tile([P, 16], FP16)
        nc.gpsimd.dma_start(out=msel_sb, in_=msel)
        ones_sb = singles.tile([1, bs], FP16)
        nc.vector.memset(ones_sb, 1.0)

        # ---- att_h = W @ h^T + b  ->  atth_sb [P, HC, bs] fp16 ----
        atth_sb = singles.tile([P, HC, bs], FP32)
        for hc in range(HC):
            a_ps_full = ps_sc.tile([P, SP], FP32, tag="sc")
            a_ps = a_ps_full[:, 0:bs]
            for dc in range(DC):
                nc.tensor.matmul(
                    a_ps,
                    lhsT=wt_sb[:, dc, hc * P : (hc + 1) * P],
                    rhs=ht_sb[:, dc, :],
                    start=(dc == 0),
                    stop=False,
                )
            nc.tensor.matmul(
                a_ps,
                lhsT=bh_sb[:, hc * P : (hc + 1) * P],
                rhs=ones_sb,
                start=False,
                stop=True,
            )
            nc.vector.tensor_copy(out=atth_sb[:, hc, :], in_=a_ps)

        def load_f(g):
            """Kick off the quartet's att_feats DMA (fp16, gpsimd/SWDGE).

            One full 128-partition transfer with 32 KB lines: partition p
            carries s-rows p and 128+p (s zero-padded to 256 host-side).
            Partial-partition DMAs run at a fraction of line rate (HW: 80
            GB/s for 68 partitions vs 375-380 GB/s for 128 with 16-32 KB
            lines), so padding 30% more bytes is a large net win, and the
            single fused DMA costs one SWDGE descriptor-emission instead
            of two.
            """
            gg = g % ngroups
            f = f1pool.tile([P, 2, G, D], FP16)
            nc.gpsimd.dma_start(out=f, in_=featsq[gg])
            return f

        def load_p(g):
            """Kick off the quartet's p_att_feats DMA (fp16, scalar queue)."""
            gg = g % ngroups
            p_sb = ppool.tile([P, G, HC * S], FP16)
            nc.scalar.dma_start(out=p_sb, in_=pTq[gg])
            return p_sb

        def a_scores(g, p_sb):
            """Bias-add + tanh + score matmuls for quartet g (scores land on
            PSUM partition 32q).  Returns the score PSUM tile."""
            sc_ps = ps_sc.tile([P, SP], FP32, tag="sc")
            nc.vector.memset(sc_ps, 0.0)
            for q in range(G):
                b = (g * G + q) % bs
                poff = 32 * q
                padd = apool.tile([P, HC, S], FP16, tag="padd")
                for hc in range(HC):
                    nc.vector.tensor_scalar_add(
                        padd[:, hc, :],
                        p_sb[:, q, hc * S : (hc + 1) * S],
                        atth_sb[:, hc, b : b + 1],
                    )
                dot = apool.tile([P, HC, S], FP16, tag="dot")
                nc.scalar.activation(out=dot, in_=padd, func=AF.Tanh, scale=1.0)
                for hc in range(HC):
                    nc.tensor.matmul(
                        sc_ps[poff : poff + 1, 0:S],
                        lhsT=wal_sb[:, hc : hc + 1],
                        rhs=dot[:, hc, :],
                        start=(hc == 0),
                        stop=(hc == HC - 1),
                        tile_position=(0, poff),
                    )
            return sc_ps

        def a_finish(g, sc_ps):
            """Masked softmax + select-transpose for quartet g.

            Returns wtT_sb [P, 2, 16] fp16: [:, 0, 4q] = batch q weights for
            s in [0,128); [:, 1, 4q] = weights for s in [128,196); all other
            columns exactly zero (mask rows != 32q are zero host-side).
            """
            gg = g % ngroups
            mx = gpool.tile([P, 1], FP32)
            nc.vector.reduce_max(mx, sc_ps, axis=AX.X)
            nm = gpool.tile([P, 1], FP32)
            nc.vector.tensor_scalar_mul(nm, mx, -1.0)
            e_sb = gpool.tile([P, SP], FP32)
            nc.scalar.activation(out=e_sb, in_=sc_ps, func=AF.Exp, bias=nm, scale=1.0)
            em = gpool.tile([P, SP], FP32)
            nc.vector.tensor_mul(em, e_sb, msk_sb[:, gg, :])
            zz = gpool.tile([P, 1], FP32)
            nc.vector.reduce_sum(zz, em, axis=AX.X)
            zze = gpool.tile([P, 1], FP32)
            nc.vector.tensor_scalar_add(zze, zz, 1e-20)
            rz = gpool.tile([P, 1], FP32)
            nc.vector.reciprocal(rz, zze)
            wgt = gpool.tile([P, SP], FP16)
            nc.vector.tensor_scalar_mul(wgt, em, rz)

            # select-transpose: wtT[s, j] = sum_r wgt[r, s] * msel[r, j]
            wtT_ps = ps_wt.tile([P, 2, 16], FP32)
            nc.tensor.matmul(
                wtT_ps[:, 0, :], lhsT=wgt[:, 0:P], rhs=msel_sb, start=True, stop=True
            )
            nc.tensor.matmul(
                wtT_ps[0 : SP - P, 1, :],
                lhsT=wgt[:, P:SP],
                rhs=msel_sb,
                start=True,
                stop=True,
            )
            wtT_sb = wtpool.tile([P, 2, 16], FP16)
            nc.vector.tensor_copy(out=wtT_sb[:, 0, :], in_=wtT_ps[:, 0, :])
            nc.vector.tensor_copy(
                out=wtT_sb[0 : SP - P, 1, :], in_=wtT_ps[0 : SP - P, 1, :]
            )
            return wtT_sb

        def phase_b_mm(g, wtT_sb, f):
            """att_res rows for quartet g: 4 batches accumulate into one
            [4, 512] PSUM region per column chunk via zero-interleaved
            weight windows wtT[:, 3q:3q+4].  The half-0 PSUM->SBUF copy
            (DVE) overlaps half-1's matmuls; the half-1 copy is deferred
            to phase_b_fin one iteration later so it never blocks the next
            quartet's prep work in the engine FIFOs."""
            row_sb = rpool.tile([G, D // 512, 512], FP32)
            res_list = []
            for half in range(2):
                res_ps = ps_res.tile([G, 2, 512], FP32)
                res_list.append(res_ps)
                for q in range(G):
                    for sc in range(2):
                        lhsT = (
                            wtT_sb[:, 0, 3 * q : 3 * q + 4]
                            if sc == 0
                            else wtT_sb[0:S1, 1, 3 * q : 3 * q + 4]
                        )
                        for c in range(2):
                            cc = 2 * half + c
                            rhs = (
                                f[:, 0, q, cc * 512 : (cc + 1) * 512]
                                if sc == 0
                                else f[0:S1, 1, q, cc * 512 : (cc + 1) * 512]
                            )
                            nc.tensor.matmul(
                                res_ps[:, c, :],
                                lhsT=lhsT,
                                rhs=rhs,
                                start=(q == 0 and sc == 0),
                                stop=(q == G - 1 and sc == 1),
                            )
                if half == 0:
                    nc.vector.tensor_copy(out=row_sb[:, 0:2, :], in_=res_ps)
            return row_sb, res_list[1]

        def phase_b_fin(g, row_sb, res_ps1):
            """Trailing half-1 copy (scalar engine) + output store (sync
            HWDGE, off the critical gpsimd input queue)."""
            gg = g % ngroups
            nc.scalar.copy(out=row_sb[:, 2:4, :], in_=res_ps1)
            nc.sync.dma_start(out=out[gg * G : (gg + 1) * G, :], in_=row_sb)

        # Software pipeline (loads lead by 2 quartets, phase A by 1):
        #   iteration it emits
        #     loads(it+2) | a_scores(it+1) | b_fin(it-1) | b_mm(it) | a_finish(it+1)
        # so the PE FIFO is [scores(it+1)][B(it)][selects(it+1)][scores(it+2)]..
        # with every operand prepared one iteration ahead, and the trailing
        # PSUM copy of B(it-1) lands on an idle engine slot instead of
        # blocking the next quartet's bias-adds/tanh.
        total = reps * ngroups
        fs = {0: load_f(0), 1: load_f(1)}
        ps = {0: load_p(0), 1: load_p(1)}
        wts = {0: a_finish(0, a_scores(0, ps.pop(0)))}
        fin = {}
        for it in range(total):
            if it + 2 < total:
                fs[it + 2] = load_f(it + 2)
                ps[it + 2] = load_p(it + 2)
            if it + 1 < total:
                sc = a_scores(it + 1, ps.pop(it + 1))
            if it >= 1:
                phase_b_fin(it - 1, *fin.pop(it - 1))
            fin[it] = phase_b_mm(it, wts.pop(it), fs.pop(it))
            if it + 1 < total:
                wts[it + 1] = a_finish(it + 1, sc)
        phase_b_fin(total - 1, *fin.pop(total - 1))

    nc.compile()
    return nc


def host_prepare(inputs, bs=BS):
    """Pre-layout full inputs into per-core in_maps (host-side, untimed)."""
    h = np.asarray(inputs["h"], dtype=np.float32)
    att_feats = np.asarray(inputs["att_feats"], dtype=np.float32)
    p = np.asarray(inputs["p_att_feats"], dtype=np.float32)
    att_masks = np.asarray(inputs["att_masks"], dtype=np.float32)
    W = np.asarray(inputs["W_h2att"], dtype=np.float32)
    b_h2att = np.asarray(inputs["b_h2att"], dtype=np.float32)
    w_alpha = np.asarray(inputs["w_alpha"], dtype=np.float32)

    n_cores = h.shape[0] // bs
    ng = bs // G

    # [P, DC, H]: WT[p, dc, h] = W[h, dc*P + p]
    WT = np.ascontiguousarray(
        W.reshape(H, DC, P).transpose(2, 1, 0).astype(np.float16)
    )
    # [P, HC]: wal[p, hc] = w_alpha[hc*P + p]
    wal = np.ascontiguousarray(w_alpha.reshape(HC, P).T.astype(np.float16))
    bhv = np.ascontiguousarray(b_h2att.reshape(1, H).astype(np.float16))
    # selection matrix: Msel[32q, 4q] = 1
    msel = np.zeros((P, 16), dtype=np.float16)
    for q in range(G):
        msel[32 * q, 4 * q] = 1.0

    in_maps = []
    for c in range(n_cores):
        b0 = c * bs
        # [ng, P, 2, G, D] fp16: partition p carries s-rows p and 128+p
        # (s zero-padded to 256) so the per-quartet feats DMA is one full
        # 128-partition transfer with 32 KB lines
        fq = np.zeros((ng, 2 * P, G, D), dtype=np.float16)
        fq[:, 0:S] = (
            att_feats[b0 : b0 + bs]
            .reshape(ng, G, S, D)
            .transpose(0, 2, 1, 3)
            .astype(np.float16)
        )
        fq = np.ascontiguousarray(fq.reshape(ng, 2, P, G, D).transpose(0, 2, 1, 3, 4))
        # [ng, P, G, HC*S] fp16: pTq[g, p, q, hc*S + s] = p[g*G+q, s, hc*P+p]
        pTq = np.ascontiguousarray(
            p[b0 : b0 + bs]
            .reshape(ng, G, S, HC, P)
            .transpose(0, 4, 1, 3, 2)
            .reshape(ng, P, G, HC * S)
            .astype(np.float16)
        )
        # [P, DC, bs] fp16: hT[p, dc, b] = h[b, dc*P + p]
        hT = np.ascontiguousarray(
            h[b0 : b0 + bs].reshape(bs, DC, P).transpose(2, 1, 0).astype(np.float16)
        )
        # [P, ng, SP] fp32: row 32q of group g = mask[g*G+q] (padded), else 0
        mT = np.zeros((P, ng, SP), dtype=np.float32)
        for q in range(G):
            mT[32 * q, :, 0:S] = att_masks[b0 : b0 + bs].reshape(ng, G, S)[:, q, :]
        in_maps.append(
            {
                "featsq": fq,
                "pTq": pTq,
                "hT": hT,
                "WT": WT,
                "walpha": wal,
                "bh": bhv,
                "masksT": np.ascontiguousarray(mT),
                "msel": msel,
            }
        )
    return in_maps


_PROGRAM = None


def _get_program():
    global _PROGRAM
    if _PROGRAM is None:
        _PROGRAM = build_program()
    return _PROGRAM


def run(inputs, trace=False):
    nc = _get_program()
    in_maps = host_prepare(inputs)
    res = run_bass_kernel_spmd(nc, in_maps, list(range(N_CORES)), trace=trace)
    out = np.concatenate([r["out"] for r in res.results], axis=0)
    return out, res


def kernel(**inputs) -> np.ndarray:
    out, _ = run(inputs, trace=False)
    return out


def _enable_ntff():
    """Best-effort: register the axon NTFF profile hook if the image's
    antenv package lacks axon_hooks.  Analysis/timing only."""
    import types

    name = "antenv.axon_hooks"
    if name in sys.modules:
        return True
    try:
        mod = types.ModuleType(name)
        state = {"hook": None}
        mod.set_axon_ntff_profile_hook = lambda h: state.__setitem__("hook", h)
        mod.get_axon_ntff_profile_hook = lambda: state["hook"]
        sys.modules[name] = mod
        import antenv

        antenv.axon_hooks = mod
        from trn_agent_boot.trn_boot import _ntff_profile_via_ctypes

        mod.set_axon_ntff_profile_hook(
            _ntff_profile_via_ctypes("/opt/axon/libaxon_pjrt.so")
        )
        return True
    except Exception:
        sys.modules.pop(name, None)
        return False


def bench(inputs, reps_long=5):
    """Device-measured timing via NTFF profiles: run the reps=1 and
    reps=reps_long programs with tracing and difference their on-device
    exec times.  The preamble (param loads + att_h) and profile framing
    cancel in the slope; the result is the steady-state time for one
    pass over the data.

    Returns (per_rep_s, t1_s, tn_s, out).
    """
    have_ntff = _enable_ntff()
    in_maps = host_prepare(inputs)
    nc1 = _get_program()
    ncn = build_program(reps=reps_long)

    def timed(nc):
        res = run_bass_kernel_spmd(
            nc, in_maps, list(range(N_CORES)), trace=have_ntff
        )
        out = np.concatenate([r["out"] for r in res.results], axis=0)
        return res.exec_time_ns, out

    t1, out = timed(nc1)
    tn, _ = timed(ncn)
    if t1 is None or tn is None:
        raise RuntimeError("NTFF timing unavailable (no exec_time_ns)")
    per_rep = (tn - t1) / (reps_long - 1) * 1e-9
    return per_rep, t1 * 1e-9, tn * 1e-9, out
